# revision 1
# baseline (speedup 1.0000x reference)
"""Trainium2 Bass kernel for nn_AdvancedGNN (2-layer GAT + BN + mean-pool + MLP).

Strategy (8 NeuronCores, SPMD, 3 launches):
  - Sort edges (with self-loops) by dst; shard dst nodes across 8 cores
    (6250 rows each), group into 128-row dst-blocks.
  - L1: every core computes h_ext1 = [x@W1 | x@W1@As | x@W1@Ad] for all
    nodes (replicated dense matmul, node-major rows in DRAM), then for its
    own dst shard gathers per-edge rows via indirect DMA, computes
    softmax-weighted scatter via one-hot matmuls accumulating in PSUM.
    Softmax skips the max-subtraction (shift-invariant; scores are small).
    Denominator is applied after accumulation (per-dst scaling).
    Outputs y1 = relu(gat1 + b1) shard + per-feature partial stats.
  - Host glue: assemble y1 (transpose only, no float math).
  - L2: BN1 is folded as a per-feature affine (a1*y + d1) applied in the
    transposed dense pass; same edge phase; pooling via matmul with a
    host-built one-hot/count matrix; outputs pooled^T partials + stats2.
  - L3 (1 core): combine partials, BN2 affine, 2-layer MLP -> [256, 2].
"""

import sys
import numpy as np

sys.path.insert(0, "/opt/trn_rl_repo")

P = 128
H = 4
HID = 32
HD = 128
NEG = 0.2
BN_EPS = 1e-5
DUMMY_AS = -1e30
ROWW = 136  # h_ext row: [h(128) | alpha_src(4) | alpha_dst(4)]


# ----------------------------------------------------------------------------
# Host-side preprocessing (index bookkeeping only)
# ----------------------------------------------------------------------------

def _prep(edge_index, batch, n, g, ncores):
    src = np.asarray(edge_index[0]).astype(np.int64)
    dst = np.asarray(edge_index[1]).astype(np.int64)
    loops = np.arange(n, dtype=np.int64)
    src = np.concatenate([src, loops])
    dst = np.concatenate([dst, loops])
    order = np.argsort(dst, kind="stable")
    src = src[order]
    dst = dst[order]

    assert n % ncores == 0
    sh = n // ncores                      # dst rows per core
    nb = (sh + P - 1) // P                # dst blocks per core
    used = [min(P, sh - b * P) for b in range(nb)]
    npad = ((n + 1) + P - 1) // P * P     # h_ext rows (dummy row = n)

    lo = np.empty((ncores, nb), np.int64)
    hi = np.empty((ncores, nb), np.int64)
    for c in range(ncores):
        for b in range(nb):
            base = c * sh + b * P
            lo[c, b] = np.searchsorted(dst, base)
            hi[c, b] = np.searchsorted(dst, base + used[b])
    cnt = hi - lo
    cb = [max(1, int(-(-cnt[:, b].max() // P))) for b in range(nb)]  # chunks/block
    tot = sum(cb) * P

    gsrc = np.full((ncores, tot), n, np.int32)
    gdl = np.zeros((ncores, tot), np.float32)
    import ml_dtypes
    dlrow = np.zeros((ncores, tot), ml_dtypes.bfloat16)  # dl, device-col order
    dstcol = np.zeros((ncores, nb * P), np.int32)
    for c in range(ncores):
        off = 0
        for b in range(nb):
            k = int(cnt[c, b])
            sl = slice(int(lo[c, b]), int(hi[c, b]))
            gsrc[c, off:off + k] = src[sl]
            gdl[c, off:off + k] = (dst[sl] - c * sh - b * P).astype(np.float32)
            off += cb[b] * P
        off = 0
        for b in range(nb):
            cpb = cb[b] * P
            # device column (c'*128 + p) <-> edge slot (p*C + c')
            dlrow[c, off:off + cpb] = gdl[c, off:off + cpb].reshape(
                P, cb[b]).T.ravel()
            off += cpb
        for b in range(nb):
            base = c * sh + b * P
            rows = np.minimum(np.arange(P) + base, base + used[b] - 1)
            dstcol[c, b * P:(b + 1) * P] = rows

    batch = np.asarray(batch).astype(np.int64)
    cnts = np.bincount(batch, minlength=g)
    w = (1.0 / np.maximum(cnts, 1)).astype(np.float32)
    spool = np.zeros((ncores, nb * P, g), np.float32)
    for c in range(ncores):
        rows = np.arange(sh)
        bids = batch[c * sh:(c + 1) * sh]
        spool[c, rows, bids] = w[bids]

    return dict(n=n, g=g, ncores=ncores, sh=sh, nb=nb, used=used, npad=npad,
                cb=cb, tot=tot, gsrc=gsrc, gdl=gdl, dlrow=dlrow, dstcol=dstcol,
                spool=spool)


def _blk_diag(a):  # [H, HID] -> [HD, H]
    out = np.zeros((HD, H), np.float32)
    for h in range(H):
        out[h * HID:(h + 1) * HID, h] = a[h]
    return out


# ----------------------------------------------------------------------------
# Numpy mirror of the device program (for validation)
# ----------------------------------------------------------------------------

def _np_edge_phase(pp, hext, core, bias):
    """Returns y=[sh,HD] (relu(gat_out+bias)), given h_ext [npad, 136]."""
    nb, cb, used, sh = pp["nb"], pp["cb"], pp["used"], pp["sh"]
    y = np.zeros((sh, HD), np.float32)
    off = 0
    for b in range(nb):
        ns = cb[b] * P
        base = core * sh + b * P
        dl_i = pp["gdl"][core, off:off + ns].astype(np.int64)
        import ml_dtypes
        es = hext[pp["gsrc"][core, off:off + ns], 128:132]
        ed = hext[base + dl_i, 132:136].astype(ml_dtypes.bfloat16).astype(
            np.float32)  # device expands ad via bf16 one-hot matmul
        hrows = hext[pp["gsrc"][core, off:off + ns], :128]
        s = es + ed
        s = np.where(s > 0, s, NEG * s)
        ex = np.exp(s)                                     # [ns, 4]
        num = np.zeros((P, HD), np.float32)
        den = np.zeros((P, H), np.float32)
        xsc = hrows.reshape(ns, H, HID) * ex[:, :, None]
        np.add.at(num, dl_i, xsc.reshape(ns, HD))
        np.add.at(den, dl_i, ex)
        out = num.reshape(P, H, HID) / (den + 1e-16)[:, :, None]
        yb = out.reshape(P, HD) + bias
        yb = np.maximum(yb, 0.0)
        y[b * P:b * P + used[b]] = yb[:used[b]]
        off += ns
    return y


def _np_forward(pp, inp):
    n, g, ncores, npad = pp["n"], pp["g"], pp["ncores"], pp["npad"]
    x = np.asarray(inp["x"], np.float32)
    As1 = _blk_diag(np.asarray(inp["a1_src"]))
    Ad1 = _blk_diag(np.asarray(inp["a1_dst"]))
    As2 = _blk_diag(np.asarray(inp["a2_src"]))
    Ad2 = _blk_diag(np.asarray(inp["a2_dst"]))
    W1, W2 = np.asarray(inp["W1"]), np.asarray(inp["W2"])

    # ---- L1
    hext1 = np.zeros((npad, ROWW), np.float32)
    h1 = x @ W1
    hext1[:n, :128] = h1
    hext1[:n, 128:132] = h1 @ As1
    hext1[:n, 132:136] = h1 @ Ad1
    hext1[n, 128:132] = DUMMY_AS
    y1 = np.zeros((n, HD), np.float32)
    st1 = np.zeros((ncores, HD, 2), np.float32)
    for c in range(ncores):
        yc = _np_edge_phase(pp, hext1, c, np.asarray(inp["b1"]))
        y1[c * pp["sh"]:(c + 1) * pp["sh"]] = yc
        st1[c, :, 0] = yc.sum(0)
        st1[c, :, 1] = (yc * yc).sum(0)

    # ---- L2
    ssum = st1.sum(0)
    mu1 = ssum[:, 0] / n
    var1 = ssum[:, 1] / n - mu1 ** 2
    a1 = np.asarray(inp["bn1_g"]) / np.sqrt(var1 + BN_EPS)
    d1 = np.asarray(inp["bn1_b"]) - a1 * mu1
    y1p = np.zeros((npad, HD), np.float32)
    y1p[:n] = y1
    h1n = a1[None, :] * y1p + d1[None, :]                  # incl pad rows (garbage ok)
    hext2 = np.zeros((npad, ROWW), np.float32)
    h2 = h1n @ W2
    hext2[:, :128] = h2
    hext2[:, 128:132] = h2 @ As2
    hext2[:, 132:136] = h2 @ Ad2
    hext2[n, 128:132] = DUMMY_AS
    poolT = np.zeros((ncores, HD, g), np.float32)
    st2 = np.zeros((ncores, HD, 2), np.float32)
    for c in range(ncores):
        yc = _np_edge_phase(pp, hext2, c, np.asarray(inp["b2"]))
        st2[c, :, 0] = yc.sum(0)
        st2[c, :, 1] = (yc * yc).sum(0)
        sp = pp["spool"][c][:pp["sh"]]
        poolT[c] = yc.T @ sp

    # ---- L3
    ssum2 = st2.sum(0)
    mu2 = ssum2[:, 0] / n
    var2 = ssum2[:, 1] / n - mu2 ** 2
    a2 = np.asarray(inp["bn2_g"]) / np.sqrt(var2 + BN_EPS)
    d2 = np.asarray(inp["bn2_b"]) - a2 * mu2
    pT = poolT.sum(0)                                      # [HD, g]
    pbnT = a2[:, None] * pT + d2[:, None]
    zT = np.asarray(inp["L1W"]).T @ pbnT + np.asarray(inp["L1b"])[:, None]
    zT = np.maximum(zT, 0.0)                               # [DOUT, g]
    out = zT.T @ np.asarray(inp["L2W"]) + np.asarray(inp["L2b"])[None, :]
    return out.astype(np.float32)                          # [g, 2]


# ----------------------------------------------------------------------------
# Bass builders
# ----------------------------------------------------------------------------

def _wext_setup(ctx, tc, ins, wkey, wtkey, askey, adkey):
    """Build W_ext [128,136] = [W | W@As | W@Ad] in SBUF. Returns the tile."""
    import concourse.tile as tile  # noqa
    from concourse import mybir
    nc = tc.nc
    f32 = mybir.dt.float32
    pool = ctx.enter_context(tc.tile_pool(name="wsetup", bufs=1))
    wext = pool.tile([P, ROWW], f32)
    wt = pool.tile([P, P], f32)
    asb = pool.tile([P, H], f32)
    adb = pool.tile([P, H], f32)
    nc.sync.dma_start(wext[:, 0:128], ins[wkey])
    nc.sync.dma_start(wt[:], ins[wtkey])
    nc.sync.dma_start(asb[:], ins[askey])
    nc.sync.dma_start(adb[:], ins[adkey])
    with tc.tile_pool(name="wsetup_ps", bufs=1, space="PSUM") as psp:
        wa = psp.tile([P, 2 * H], f32)
        nc.tensor.matmul(wa[:, 0:H], lhsT=wt[:], rhs=asb[:], start=True, stop=True)
        nc.tensor.matmul(wa[:, H:2 * H], lhsT=wt[:], rhs=adb[:], start=True, stop=True)
        nc.vector.tensor_copy(wext[:, 128:136], wa[:])
    return wext


def _edge_phase(ctx, tc, cfg, ins, hext_ap, wrap_tail):
    """Shared edge phase. wrap_tail(b, used, yb_ap, pools) handles per-block output."""
    import concourse.tile as tile  # noqa
    from concourse import mybir, bass
    nc = tc.nc
    f32, i32 = mybir.dt.float32, mybir.dt.int32
    bff = mybir.dt.bfloat16
    nb, cbs, used = cfg["nb"], cfg["cb"], cfg["used"]
    cmax = max(cbs)

    pool = ctx.enter_context(tc.tile_pool(name="edge_sb", bufs=2))
    spool = ctx.enter_context(tc.tile_pool(name="edge_small", bufs=2))
    psp = ctx.enter_context(tc.tile_pool(name="edge_ps", bufs=2, space="PSUM"))

    # constants
    cpool = ctx.enter_context(tc.tile_pool(name="edge_const", bufs=1))
    iota_i = cpool.tile([P, P], i32)
    nc.gpsimd.iota(iota_i[:], pattern=[[1, P]], base=0, channel_multiplier=0)
    iota_f = cpool.tile([P, P], f32)
    nc.vector.tensor_copy(iota_f[:], iota_i[:])
    pj_i = cpool.tile([P, 1], i32)
    nc.gpsimd.iota(pj_i[:], pattern=[[0, 1]], base=0, channel_multiplier=1)
    pj_col = cpool.tile([P, 1], f32)
    nc.vector.tensor_copy(pj_col[:], pj_i[:])
    bias_b = cpool.tile([P, HD], f32)
    nc.sync.dma_start(bias_b[:], ins[cfg["bias_key"]][None, :].to_broadcast([P, HD]))
    ones_t = cpool.tile([P, 1], f32)
    nc.vector.memset(ones_t[:], 1.0)

    off = 0
    for b in range(nb):
        cb = cbs[b]
        ns = cb * P
        idx = spool.tile([P, cmax], i32, tag="idx")
        nc.sync.dma_start(idx[:, :cb],
                          ins["gsrc"][off:off + ns].rearrange("(p c) -> p c", c=cb))
        ddl = spool.tile([P, cmax], f32, tag="ddl")
        nc.sync.dma_start(ddl[:, :cb],
                          ins["gdl"][off:off + ns].rearrange("(p c) -> p c", c=cb))

        gt = pool.tile([P, cmax * 132], f32, tag="gt")
        gt3 = gt[:].rearrange("p (c e) -> p c e", e=132)[:, :cb]
        for c in range(cb):
            # HW indirect DMA honors one index per partition -> per-chunk op
            nc.gpsimd.indirect_dma_start(
                out=gt3[:, c], out_offset=None, in_=hext_ap,
                in_offset=bass.IndirectOffsetOnAxis(ap=idx[:, c:c + 1],
                                                    axis=0))

        # ad[dst] per edge = S^T(host, bf16) @ ad_rows-of-block
        dci = spool.tile([P, 1], i32, tag="dci")
        nc.sync.dma_start(dci[:], ins["dstcol"][b * P:(b + 1) * P, None])
        adl = spool.tile([P, H], f32, tag="adl")
        nc.gpsimd.indirect_dma_start(
            out=adl[:], out_offset=None, in_=hext_ap,
            in_offset=bass.IndirectOffsetOnAxis(ap=dci[:], axis=0),
            element_offset=132)
        adlb = spool.tile([P, H], bff, tag="adlb")
        nc.vector.tensor_copy(adlb[:], adl[:])
        dlb_t = pool.tile([P, cmax * P], bff, tag="dlb_t")
        nc.scalar.dma_start(dlb_t[:, :cb * P],
                            ins["dlrow"][None, off:off + ns].to_broadcast([P, ns]))
        stht = pool.tile([P, cmax * P], bff, tag="stht")
        nc.vector.tensor_scalar(out=stht[:, :cb * P], in0=dlb_t[:, :cb * P],
                                scalar1=pj_col[:], scalar2=None,
                                op0=mybir.AluOpType.is_equal)
        stht3 = stht[:].rearrange("p (c e) -> p c e", e=P)[:, :cb]
        edp = psp.tile([P, cmax * H], f32, tag="edp", bufs=2)
        for c in range(cb):
            nc.tensor.matmul(edp[:, c * H:(c + 1) * H], lhsT=stht3[:, c],
                             rhs=adlb[:], start=True, stop=True)
        ed3 = edp[:].rearrange("p (c e) -> p c e", e=H)[:, :cb]

        # scores -> ex, stored into xe[:, :, 128:132]
        xe = pool.tile([P, cmax * 132], f32, tag="xe")
        xe3 = xe[:].rearrange("p (c e) -> p c e", e=132)[:, :cb]
        st = spool.tile([P, cmax * H], f32, tag="st")
        st3 = st[:].rearrange("p (c e) -> p c e", e=H)[:, :cb]
        nc.vector.tensor_tensor(out=st3, in0=gt3[:, :, 128:132], in1=ed3,
                                op=mybir.AluOpType.add)
        sn = spool.tile([P, cmax * H], f32, tag="sn")
        sn3 = sn[:].rearrange("p (c e) -> p c e", e=H)[:, :cb]
        nc.vector.tensor_scalar(out=sn3, in0=st3, scalar1=NEG, scalar2=None,
                                op0=mybir.AluOpType.mult)
        nc.vector.tensor_tensor(out=st3, in0=st3, in1=sn3,
                                op=mybir.AluOpType.max)
        nc.scalar.activation(out=xe3[:, :, 128:132], in_=st3,
                             func=mybir.ActivationFunctionType.Exp)

        # xe[:, :, h*32:(h+1)*32] = h_gath * ex_h
        for h in range(H):
            nc.vector.tensor_tensor(
                out=xe3[:, :, h * HID:(h + 1) * HID],
                in0=gt3[:, :, h * HID:(h + 1) * HID],
                in1=xe3[:, :, 128 + h:129 + h].to_broadcast([P, cb, HID]),
                op=mybir.AluOpType.mult)

        # one-hot S [e, dst_local]
        sb = pool.tile([P, cmax * P], f32, tag="sb")
        sb3 = sb[:].rearrange("p (c e) -> p c e", e=P)[:, :cb]
        nc.vector.tensor_tensor(
            out=sb3,
            in0=iota_f[:, None, :].to_broadcast([P, cb, P]),
            in1=ddl[:, :cb][:, :, None].to_broadcast([P, cb, P]),
            op=mybir.AluOpType.is_equal)

        ps = psp.tile([P, 132], f32, tag="ps")
        for c in range(cb):
            nc.tensor.matmul(ps[:], lhsT=sb3[:, c], rhs=xe3[:, c],
                             start=(c == 0), stop=(c == cb - 1))

        den = spool.tile([P, H], f32, tag="den")
        nc.vector.tensor_scalar(out=den[:], in0=ps[:, 128:132], scalar1=1e-16,
                                scalar2=None, op0=mybir.AluOpType.add)
        nc.vector.reciprocal(den[:], den[:])
        yb = spool.tile([P, HD], f32, tag="yb")
        for h in range(H):
            nc.vector.tensor_scalar(out=yb[:, h * HID:(h + 1) * HID],
                                    in0=ps[:, h * HID:(h + 1) * HID],
                                    scalar1=den[:, h:h + 1], scalar2=None,
                                    op0=mybir.AluOpType.mult)
        nc.vector.tensor_tensor(out=yb[:], in0=yb[:], in1=bias_b[:],
                                op=mybir.AluOpType.add)
        nc.scalar.activation(out=yb[:], in_=yb[:],
                             func=mybir.ActivationFunctionType.Relu)
        wrap_tail(b, used[b], yb, dict(spool=spool, psp=psp, ones=ones_t))
        off += ns


def _dense_phase(ctx, tc, cfg, src_ap, wext, hext_ap, affine=None, psum_bufs=2):
    """h_ext[npad,136] = (affine(src^T))^T @ wext, written chunk by chunk."""
    from concourse import mybir
    nc = tc.nc
    f32 = mybir.dt.float32
    nch = cfg["npad"] // P
    pool = ctx.enter_context(tc.tile_pool(name="dense_sb", bufs=3))
    psp = ctx.enter_context(tc.tile_pool(name="dense_ps", bufs=psum_bufs, space="PSUM"))
    hext3 = hext_ap.rearrange("(g p) e -> p g e", p=P)
    for gi, c0 in enumerate(range(0, nch, 2)):
        w = min(2, nch - c0)
        xt = pool.tile([P, 2 * P], f32, tag="xt")
        eng = nc.sync if gi % 2 == 0 else nc.scalar
        eng.dma_start(xt[:, :w * P], src_ap[:, c0 * P:(c0 + w) * P])
        if affine is not None:
            a_col, d_col = affine
            nc.vector.tensor_scalar(out=xt[:, :w * P], in0=xt[:, :w * P],
                                    scalar1=a_col, scalar2=d_col,
                                    op0=mybir.AluOpType.mult,
                                    op1=mybir.AluOpType.add)
        ps = psp.tile([P, 2 * ROWW], f32, tag="dps")
        for k in range(w):
            nc.tensor.matmul(ps[:, k * ROWW:(k + 1) * ROWW],
                             lhsT=xt[:, k * P:(k + 1) * P], rhs=wext[:],
                             start=True, stop=True)
        hx = pool.tile([P, 2 * ROWW], f32, tag="hx")
        nc.vector.tensor_copy(hx[:, :w * ROWW], ps[:, :w * ROWW])
        eng2 = nc.scalar if gi % 2 == 0 else nc.sync
        eng2.dma_start(hext3[:, c0:c0 + w, :],
                       hx[:, :w * ROWW].rearrange("p (g e) -> p g e", g=w))
    # dummy row: alpha_src = DUMMY_AS
    dmy = pool.tile([1, H], f32, tag="dmy")
    nc.vector.memset(dmy[:], DUMMY_AS)
    nc.sync.dma_start(hext_ap[cfg["n"]:cfg["n"] + 1, 128:132], dmy[:])


def _build_l1(ctx, tc, outs, ins, cfg):
    from concourse import mybir
    nc = tc.nc
    f32 = mybir.dt.float32
    wext = _wext_setup(ctx, tc, ins, "W1", "W1T", "as1", "ad1")
    dram = ctx.enter_context(tc.tile_pool(name="dram", bufs=1, space="DRAM"))
    hext = dram.tile([cfg["npad"], ROWW], f32)
    _dense_phase(ctx, tc, cfg, ins["xT"], wext[:], hext[:])

    acc_pool = ctx.enter_context(tc.tile_pool(name="acc", bufs=1))
    st_acc = acc_pool.tile([P, 2], f32)
    nc.vector.memset(st_acc[:], 0.0)

    def tail(b, used, yb, pools):
        nc.sync.dma_start(outs["y1s"][b * P:b * P + used, :], yb[:used, :])
        sq = pools["spool"].tile([P, HD], f32, tag="sq")
        nc.scalar.activation(out=sq[:], in_=yb[:],
                             func=mybir.ActivationFunctionType.Square)
        ps = pools["psp"].tile([P, 2], f32, tag="stps", bufs=1)
        nc.tensor.matmul(ps[:, 0:1], lhsT=yb[:used, :], rhs=pools["ones"][:used, :],
                         start=True, stop=True)
        nc.tensor.matmul(ps[:, 1:2], lhsT=sq[:used, :], rhs=pools["ones"][:used, :],
                         start=True, stop=True)
        nc.vector.tensor_tensor(out=st_acc[:], in0=st_acc[:], in1=ps[:],
                                op=mybir.AluOpType.add)

    cfg2 = dict(cfg, bias_key="b1")
    _edge_phase(ctx, tc, cfg2, ins, hext[:], tail)
    nc.sync.dma_start(outs["st1"][:, :], st_acc[:])


def _bn_affine(ctx, tc, ins, stkey, gkey, bkey, n, ncores):
    """Combine per-core stats -> (a_col, d_col) [128,1] tiles."""
    from concourse import mybir
    nc = tc.nc
    f32 = mybir.dt.float32
    pool = ctx.enter_context(tc.tile_pool(name="bnsetup", bufs=1))
    acc = pool.tile([P, 2], f32)
    tmp = pool.tile([P, 2], f32)
    nc.sync.dma_start(acc[:], ins[stkey][0])
    for c in range(1, ncores):
        nc.sync.dma_start(tmp[:], ins[stkey][c])
        nc.vector.tensor_tensor(out=acc[:], in0=acc[:], in1=tmp[:],
                                op=mybir.AluOpType.add)
    mu = pool.tile([P, 1], f32)
    nc.vector.tensor_scalar(out=mu[:], in0=acc[:, 0:1], scalar1=1.0 / n,
                            scalar2=None, op0=mybir.AluOpType.mult)
    var = pool.tile([P, 1], f32)
    nc.vector.tensor_scalar(out=var[:], in0=acc[:, 1:2], scalar1=1.0 / n,
                            scalar2=None, op0=mybir.AluOpType.mult)
    musq = pool.tile([P, 1], f32)
    nc.scalar.activation(out=musq[:], in_=mu[:],
                         func=mybir.ActivationFunctionType.Square)
    nc.vector.tensor_tensor(out=var[:], in0=var[:], in1=musq[:],
                            op=mybir.AluOpType.subtract)
    nc.vector.tensor_scalar(out=var[:], in0=var[:], scalar1=BN_EPS,
                            scalar2=None, op0=mybir.AluOpType.add)
    sd = pool.tile([P, 1], f32)
    nc.scalar.activation(out=sd[:], in_=var[:],
                         func=mybir.ActivationFunctionType.Sqrt)
    rs = pool.tile([P, 1], f32)
    nc.vector.reciprocal(rs[:], sd[:])
    gc = pool.tile([P, 1], f32)
    nc.sync.dma_start(gc[:], ins[gkey][:, None])
    bc = pool.tile([P, 1], f32)
    nc.sync.dma_start(bc[:], ins[bkey][:, None])
    a_col = pool.tile([P, 1], f32)
    nc.vector.tensor_tensor(out=a_col[:], in0=gc[:], in1=rs[:],
                            op=mybir.AluOpType.mult)
    d_col = pool.tile([P, 1], f32)
    nc.vector.tensor_tensor(out=d_col[:], in0=a_col[:], in1=mu[:],
                            op=mybir.AluOpType.mult)
    nc.vector.tensor_tensor(out=d_col[:], in0=bc[:], in1=d_col[:],
                            op=mybir.AluOpType.subtract)
    return a_col, d_col


def _build_l2(ctx, tc, outs, ins, cfg):
    from concourse import mybir
    nc = tc.nc
    f32 = mybir.dt.float32
    g = cfg["g"]
    wext = _wext_setup(ctx, tc, ins, "W2", "W2T", "as2", "ad2")
    a_col, d_col = _bn_affine(ctx, tc, ins, "st1_all", "bn1_g", "bn1_b",
                              cfg["n"], cfg["ncores"])
    dram = ctx.enter_context(tc.tile_pool(name="dram", bufs=1, space="DRAM"))
    hext = dram.tile([cfg["npad"], ROWW], f32)
    _dense_phase(ctx, tc, cfg, ins["y1T"], wext[:], hext[:],
                 affine=(a_col[:], d_col[:]), psum_bufs=1)

    acc_pool = ctx.enter_context(tc.tile_pool(name="acc", bufs=1))
    st_acc = acc_pool.tile([P, 2], f32)
    nc.vector.memset(st_acc[:], 0.0)
    pool_acc = acc_pool.tile([P, g], f32)
    nc.vector.memset(pool_acc[:], 0.0)
    pl_ps = ctx.enter_context(tc.tile_pool(name="plps", bufs=2, space="PSUM"))
    sp_pool = ctx.enter_context(tc.tile_pool(name="spt", bufs=2))

    def tail(b, used, yb, pools):
        sq = pools["spool"].tile([P, HD], f32, tag="sq")
        nc.scalar.activation(out=sq[:], in_=yb[:],
                             func=mybir.ActivationFunctionType.Square)
        ps = pools["psp"].tile([P, 2], f32, tag="stps", bufs=1)
        nc.tensor.matmul(ps[:, 0:1], lhsT=yb[:used, :], rhs=pools["ones"][:used, :],
                         start=True, stop=True)
        nc.tensor.matmul(ps[:, 1:2], lhsT=sq[:used, :], rhs=pools["ones"][:used, :],
                         start=True, stop=True)
        nc.vector.tensor_tensor(out=st_acc[:], in0=st_acc[:], in1=ps[:],
                                op=mybir.AluOpType.add)
        spt = sp_pool.tile([P, g], f32, tag="spt")
        nc.sync.dma_start(spt[:], ins["spool"][b * P:(b + 1) * P, :])
        pps = pl_ps.tile([P, g], f32, tag="pps")
        nc.tensor.matmul(pps[:], lhsT=yb[:used, :], rhs=spt[:used, :],
                         start=True, stop=True)
        nc.vector.tensor_tensor(out=pool_acc[:], in0=pool_acc[:], in1=pps[:],
                                op=mybir.AluOpType.add)

    cfg2 = dict(cfg, bias_key="b2")
    _edge_phase(ctx, tc, cfg2, ins, hext[:], tail)
    nc.sync.dma_start(outs["st2"][:, :], st_acc[:])
    nc.sync.dma_start(outs["poolT"][:, :], pool_acc[:])


def _build_l3(ctx, tc, outs, ins, cfg):
    from concourse import mybir
    nc = tc.nc
    f32 = mybir.dt.float32
    g, ncores, n = cfg["g"], cfg["ncores"], cfg["n"]
    pool = ctx.enter_context(tc.tile_pool(name="l3", bufs=1))
    psp = ctx.enter_context(tc.tile_pool(name="l3ps", bufs=1, space="PSUM"))

    a2, d2 = _bn_affine(ctx, tc, ins, "st2_all", "bn2_g", "bn2_b", n, ncores)
    pt = pool.tile([P, g], f32)
    tmp = pool.tile([P, g], f32)
    nc.sync.dma_start(pt[:], ins["poolT_all"][0])
    for c in range(1, ncores):
        nc.sync.dma_start(tmp[:], ins["poolT_all"][c])
        nc.vector.tensor_tensor(out=pt[:], in0=pt[:], in1=tmp[:],
                                op=mybir.AluOpType.add)
    nc.vector.tensor_scalar(out=pt[:], in0=pt[:], scalar1=a2[:], scalar2=d2[:],
                            op0=mybir.AluOpType.mult, op1=mybir.AluOpType.add)
    l1w = pool.tile([P, P], f32)
    nc.sync.dma_start(l1w[:], ins["L1W"])
    l1b = pool.tile([P, 1], f32)
    nc.sync.dma_start(l1b[:], ins["L1b"][:, None])
    zps = psp.tile([P, g], f32)
    nc.tensor.matmul(zps[:], lhsT=l1w[:], rhs=pt[:], start=True, stop=True)
    zt = pool.tile([P, g], f32)
    nc.scalar.activation(out=zt[:], in_=zps[:],
                         func=mybir.ActivationFunctionType.Relu, bias=l1b[:])
    l2w = pool.tile([P, 2], f32)
    nc.sync.dma_start(l2w[:], ins["L2W"])
    l2bb = pool.tile([P, 2], f32)
    nc.sync.dma_start(l2bb[:], ins["L2b"][None, :].to_broadcast([P, 2]))
    for hf in range((g + P - 1) // P):
        hsz = min(P, g - hf * P)
        ops = psp.tile([P, 2], f32, tag="ops", bufs=2)
        nc.tensor.matmul(ops[:hsz], lhsT=zt[:, hf * P:hf * P + hsz], rhs=l2w[:],
                         start=True, stop=True)
        ob = pool.tile([P, 2], f32, tag="ob", bufs=2)
        nc.vector.tensor_tensor(out=ob[:hsz], in0=ops[:hsz], in1=l2bb[:hsz],
                                op=mybir.AluOpType.add)
        nc.sync.dma_start(outs["out"][hf * P:hf * P + hsz, :], ob[:hsz])


# ----------------------------------------------------------------------------
# Launch runner
# ----------------------------------------------------------------------------

def _install_trace_shims():
    """Make trace=True work in this image: provide the missing
    antenv.axon_hooks module and neuter the artifact upload."""
    import types
    from concourse import bass_utils
    try:
        from antenv.axon_hooks import get_axon_ntff_profile_hook  # noqa: F401
    except ImportError:
        from trn_agent_boot.trn_boot import _ntff_profile_via_ctypes
        hook = _ntff_profile_via_ctypes("/opt/axon/libaxon_pjrt.so")
        m = types.ModuleType("antenv.axon_hooks")
        m.get_axon_ntff_profile_hook = lambda: hook
        m.set_axon_ntff_profile_hook = lambda h: None
        sys.modules["antenv.axon_hooks"] = m
    bass_utils.upload_artifacts = lambda tmpdir: tmpdir


def _run_launch(builder, cfg, in_specs, out_specs, in_maps, ncores, trace=False):
    from contextlib import ExitStack
    import concourse.tile as tile
    import concourse.bass_interp as bass_interp
    from concourse import bacc, mybir, bass_utils
    if trace:
        _install_trace_shims()

    # capture the scheduling cost-model makespan (modeled kernel ns)
    sim_times = []
    orig_sim = bass_interp.CoreSim.simulate

    def patched(selfsim, *a, **k):
        r = orig_sim(selfsim, *a, **k)
        try:
            sim_times.append(int(selfsim.time))
        except Exception:
            pass
        return r

    bass_interp.CoreSim.simulate = patched

    nc = bacc.Bacc("TRN2", target_bir_lowering=False, debug=False,
                   num_devices=ncores)
    ins = {k: nc.dram_tensor(k, list(v.shape), mybir.dt.from_np(v.dtype),
                             kind="ExternalInput").ap()
           for k, v in in_specs.items()}
    outs = {k: nc.dram_tensor(k, list(shp), mybir.dt.from_np(np.dtype(dt)),
                              kind="ExternalOutput").ap()
            for k, (shp, dt) in out_specs.items()}
    try:
        with tile.TileContext(nc) as tc:
            with ExitStack() as ctx:
                builder(ctx, tc, outs, ins, cfg)
    finally:
        bass_interp.CoreSim.simulate = orig_sim
    nc.compile()
    res = bass_utils.run_bass_kernel_spmd(
        nc, in_maps, core_ids=list(range(ncores)), trace=trace)
    res.modeled_ns = max(sim_times) if sim_times else None
    return res


def kernel(**inputs):
    import os
    n = int(inputs["x"].shape[0])
    g = 256
    ncores = 8
    pp = _prep(inputs["edge_index"], inputs["batch"], n, g, ncores)
    npad, tot, nb = pp["npad"], pp["tot"], pp["nb"]
    f32 = np.float32
    cfg = dict(n=n, g=g, ncores=ncores, sh=pp["sh"], nb=nb, used=pp["used"],
               npad=npad, cb=pp["cb"], tot=tot)

    trace = os.environ.get("GNN_TRACE", "0") == "1"

    # ---------------- L1
    x = np.asarray(inputs["x"], f32)
    xT = np.zeros((P, npad), f32)
    xT[:, :n] = x.T
    in_specs1 = dict(
        xT=xT, W1=np.asarray(inputs["W1"], f32),
        W1T=np.ascontiguousarray(np.asarray(inputs["W1"], f32).T),
        as1=_blk_diag(np.asarray(inputs["a1_src"])),
        ad1=_blk_diag(np.asarray(inputs["a1_dst"])),
        b1=np.asarray(inputs["b1"], f32),
        gsrc=pp["gsrc"][0], gdl=pp["gdl"][0], dlrow=pp["dlrow"][0],
        dstcol=pp["dstcol"][0],
    )
    out_specs1 = dict(y1s=((nb * P, HD), f32), st1=((P, 2), f32))
    in_maps1 = [dict(in_specs1, gsrc=pp["gsrc"][c], gdl=pp["gdl"][c],
                     dlrow=pp["dlrow"][c], dstcol=pp["dstcol"][c])
                for c in range(ncores)]
    r1 = _run_launch(_build_l1, cfg, in_specs1, out_specs1, in_maps1, ncores,
                     trace)

    # glue: assemble y1 full, transpose
    y1T = np.zeros((P, npad), f32)
    sh = pp["sh"]
    for c in range(ncores):
        y1T[:, c * sh:(c + 1) * sh] = r1.results[c]["y1s"][:sh].T
    st1_all = np.stack([r1.results[c]["st1"] for c in range(ncores)])

    # ---------------- L2
    in_specs2 = dict(
        y1T=y1T, W2=np.asarray(inputs["W2"], f32),
        W2T=np.ascontiguousarray(np.asarray(inputs["W2"], f32).T),
        as2=_blk_diag(np.asarray(inputs["a2_src"])),
        ad2=_blk_diag(np.asarray(inputs["a2_dst"])),
        b2=np.asarray(inputs["b2"], f32),
        st1_all=st1_all,
        bn1_g=np.asarray(inputs["bn1_g"], f32),
        bn1_b=np.asarray(inputs["bn1_b"], f32),
        spool=pp["spool"][0],
        gsrc=pp["gsrc"][0], gdl=pp["gdl"][0], dlrow=pp["dlrow"][0],
        dstcol=pp["dstcol"][0],
    )
    out_specs2 = dict(poolT=((P, g), f32), st2=((P, 2), f32))
    in_maps2 = [dict(in_specs2, gsrc=pp["gsrc"][c], gdl=pp["gdl"][c],
                     dlrow=pp["dlrow"][c], dstcol=pp["dstcol"][c],
                     spool=pp["spool"][c])
                for c in range(ncores)]
    r2 = _run_launch(_build_l2, cfg, in_specs2, out_specs2, in_maps2, ncores,
                     trace)

    poolT_all = np.stack([r2.results[c]["poolT"] for c in range(ncores)])
    st2_all = np.stack([r2.results[c]["st2"] for c in range(ncores)])

    # ---------------- L3
    in_specs3 = dict(
        poolT_all=poolT_all, st2_all=st2_all,
        bn2_g=np.asarray(inputs["bn2_g"], f32),
        bn2_b=np.asarray(inputs["bn2_b"], f32),
        L1W=np.asarray(inputs["L1W"], f32),
        L1b=np.asarray(inputs["L1b"], f32),
        L2W=np.asarray(inputs["L2W"], f32),
        L2b=np.asarray(inputs["L2b"], f32),
    )
    out_specs3 = dict(out=((g, 2), f32))
    r3 = _run_launch(_build_l3, cfg, in_specs3, out_specs3, [in_specs3], 1,
                     trace)

    kernel.last_exec_ns = [r.exec_time_ns for r in (r1, r2, r3)]
    kernel.last_modeled_ns = [r.modeled_ns for r in (r1, r2, r3)]
    return r3.results[0]["out"]

